# revision 19
# baseline (speedup 1.0000x reference)
"""Causal self-attention (RoPE) Trainium2 kernel.

Distribution: 8 cores = 2 data-parallel groups (batch dim, B=2) x 4
tensor-parallel cores (16 heads -> 4 heads/core).  Each core computes
QKV projection + RoPE + causal attention + output projection for its
batch and heads; an AllGather over each 4-core group shares y so each
core computes a 512-column shard of the output projection, which the
host reassembles.

Pipelined at token-quarter granularity: QKV(q) -> attention band(q) ->
AllGather(q), so collectives overlap compute and the PE never idles.
All bulk inputs are host-pre-staged into SBUF-native layouts so each
load is a single large contiguous DMA (sync-queue issue cost ~0.6us
per dma_start makes many small DMAs expensive).

Self-contained: hardcodes all shapes from the problem spec.
"""

import numpy as np

B, T, C = 2, 2048, 2048
H, D = 16, 128
HL = 4            # heads per core
W_LOC = HL * D    # 512 local head width
NCORES = 8
GROUPS = [[0, 1, 2, 3], [4, 5, 6, 7]]
SCALE = 1.0 / float(np.sqrt(D))
NCC = C // 128    # 16 contraction chunks
QT = 512          # token quarter

_CACHE = {}


def _host_tables():
    # Mirror reference _rope_tables in float32.
    inv_freq = (1.0 / (10000.0 ** (np.arange(0, D, 2, dtype=np.float32) / np.float32(D)))).astype(np.float32)
    t = np.arange(T, dtype=np.float32)
    freqs = np.outer(t, inv_freq).astype(np.float32)        # (T, D/2)
    emb = np.concatenate([freqs, freqs], axis=-1)           # (T, D)
    cos_t = np.ascontiguousarray(np.cos(emb).astype(np.float32).T)  # (D, T)
    sin_t = np.ascontiguousarray(np.sin(emb).astype(np.float32).T)
    return cos_t, sin_t


def _host_masks():
    import ml_dtypes
    # S^T-layout causal masks for the 4 diagonal phases, pre-staged as
    # [kk, phase, qq] so one DMA loads all four.
    # mask[p][kk, qq] = 1 if qq >= kk + p*128 else 0
    kk = np.arange(128)[:, None]
    qq = np.arange(512)[None, :]
    m = np.stack([(qq >= kk + p * 128) for p in range(4)]).astype(np.float32)
    m = np.ascontiguousarray(m.transpose(1, 0, 2))          # (128, 4, 512)
    return m.astype(ml_dtypes.bfloat16)


def _host_s2():
    # Signed/swap-ready sin table for rotate_half on the vector engine:
    # t1[d] = ps[(d+64)%128] * s2[d] with s2 = [-sin[0:64]; sin[64:128]].
    _, sin_t = _host_tables()
    s2 = sin_t.copy()
    s2[0:64, :] = -s2[0:64, :]
    return np.ascontiguousarray(s2)


def _build():
    if "nc" in _CACHE:
        return _CACHE["nc"]

    import concourse.mybir as mybir
    import concourse.tile as tile
    from concourse import bacc

    f32 = mybir.dt.float32
    f32r = mybir.dt.float32r
    bf16 = mybir.dt.bfloat16

    nc = bacc.Bacc(None, target_bir_lowering=False, num_devices=NCORES)

    # Host-pre-staged layouts: partition dim first, contiguous per line.
    xq = nc.dram_tensor("xq", [4, 128, NCC, QT], bf16, kind="ExternalInput")
    wk_in = nc.dram_tensor("wk", [128, HL, NCC, 128], bf16, kind="ExternalInput")
    wq_in = nc.dram_tensor("wq", [128, HL, NCC, 128], bf16, kind="ExternalInput")
    wv_in = nc.dram_tensor("wv", [128, NCC, W_LOC], bf16, kind="ExternalInput")
    wp_in = nc.dram_tensor("wp", [128, 16, W_LOC], bf16, kind="ExternalInput")
    cos_in = nc.dram_tensor("cos_t", [D, T], f32, kind="ExternalInput")
    s2_in = nc.dram_tensor("s2_t", [D, T], f32, kind="ExternalInput")
    masks = nc.dram_tensor("masks", [128, 4, 512], bf16, kind="ExternalInput")
    identb_in = nc.dram_tensor("identb", [128, 128], bf16, kind="ExternalInput")
    out_ext = nc.dram_tensor("out_shard", [T, W_LOC], f32, kind="ExternalOutput")

    with tile.TileContext(nc) as tc:
        with (
            tc.tile_pool(name="const", bufs=1) as constp,
            tc.tile_pool(name="pers", bufs=1) as pers,
            tc.tile_pool(name="xp", bufs=2) as xp,
            tc.tile_pool(name="qp", bufs=1) as qp,
            tc.tile_pool(name="ropet", bufs=2) as ropet,
            tc.tile_pool(name="ptp", bufs=4) as ptp,
            tc.tile_pool(name="ynp", bufs=2) as ynp,
            tc.tile_pool(name="ytbp", bufs=2) as ytbp,
            tc.tile_pool(name="yagp", bufs=2) as yagp,
            tc.tile_pool(name="outp", bufs=1) as outp,
            tc.tile_pool(name="psmm", bufs=3, space="PSUM") as psmm,
            tc.tile_pool(name="pss", bufs=2, space="PSUM") as pss,
            tc.tile_pool(name="psyp", bufs=3, space="PSUM") as psyp,
            tc.tile_pool(name="dram", bufs=1, space="DRAM") as dram,
        ):
            # bands 0-2 gather all 4 heads at once; band 3 is split in two
            # (heads 0-1 / heads 2-3) so its AllGather starts mid-band.
            yag_in = [dram.tile([512, 512], bf16, name=f"yagin{qb}")
                      for qb in range(3)]
            yag_in3 = [dram.tile([256, 512], bf16, name=f"yagin3{i}")
                       for i in range(2)]
            yag_out = [dram.tile([2048, 512], bf16, name=f"yagout{qb}")
                       for qb in range(3)]
            yag_out3 = [dram.tile([1024, 512], bf16, name=f"yagout3{i}")
                        for i in range(2)]

            def load_x_quarter(qq):
                t = xp.tile([128, NCC, QT], bf16, tag="xt", name=f"xt{qq}")
                # 4 sub-DMAs so the first chunks land (and matmuls start)
                # before the whole 2MB quarter transfers.
                for i in range(4):
                    nc.sync.dma_start(
                        out=t[:, 4 * i:4 * i + 4, :],
                        in_=xq[qq, :, 4 * i:4 * i + 4, :])
                return t

            # quarter-0 activations and k-weights first: the first compute
            # (k streams) becomes runnable as soon as these land.
            xt_cur = load_x_quarter(0)
            wk_sb = constp.tile([128, HL, NCC, 128], bf16)
            nc.sync.dma_start(out=wk_sb, in_=wk_in[:, :, :, :])
            wq_sb = constp.tile([128, HL, NCC, 128], bf16)
            nc.sync.dma_start(out=wq_sb, in_=wq_in[:, :, :, :])
            cos_sb = constp.tile([D, T], f32)
            s2_sb = constp.tile([D, T], f32)
            nc.sync.dma_start(out=s2_sb[:, 0:T // 2], in_=s2_in[:, 0:T // 2])
            nc.sync.dma_start(out=cos_sb[:, 0:T // 2], in_=cos_in[:, 0:T // 2])
            wv_sb = constp.tile([128, NCC, W_LOC], bf16)
            nc.sync.dma_start(out=wv_sb, in_=wv_in[:, :, :])
            mask_sb = constp.tile([128, 4, 512], bf16)
            nc.sync.dma_start(out=mask_sb, in_=masks[:, :, :])
            identb = constp.tile([128, 128], bf16)
            nc.sync.dma_start(out=identb, in_=identb_in[:, :])

            # Persistent activations: k^T full-T per head, v natural.
            k_t = []
            for h in range(HL):
                k_t.append(pers.tile([D, T], bf16, name=f"kt{h}"))
            v_nat = []
            for tt in range(T // 128):
                vt = pers.tile([128, HL, D + 1], bf16, name=f"vnat{tt}")
                nc.vector.memset(vt[:, :, D:D + 1], 1.0)
                v_nat.append(vt)

            def rope_into(dest, ps, tg, nm):
                # rotate_half on the DVE: two half-partition muls against the
                # sign-folded sin table (s2), keeping the PE free.
                t1 = ropet.tile([128, QT], bf16, tag="t1", name=f"t1_{nm}")
                nc.vector.tensor_mul(t1[0:64, :], ps[64:128, :],
                                     s2_sb[0:64, tg:tg + QT])
                nc.vector.tensor_mul(t1[64:128, :], ps[0:64, :],
                                     s2_sb[64:128, tg:tg + QT])
                t2 = ropet.tile([128, QT], f32, tag="t2", name=f"t2_{nm}")
                nc.vector.tensor_mul(t2, ps, cos_sb[:, tg:tg + QT])
                nc.vector.tensor_add(dest, t2, t1)

            def band(qb, q_cur, after_h=None):
                nkb = 4 * (qb + 1)

                def s_exp(h, kb):
                    ps_s = pss.tile([128, 512], f32, tag="s",
                                    name=f"s{qb}_{h}_{kb}")
                    nc.tensor.matmul(
                        ps_s,
                        lhsT=k_t[h][:, kb * 128:(kb + 1) * 128],
                        rhs=q_cur[h],
                    )
                    pt = ptp.tile([128, 512], bf16, tag="pt",
                                  name=f"pt{qb}_{h}_{kb}")
                    nc.scalar.activation(
                        pt, ps_s, mybir.ActivationFunctionType.Exp,
                        scale=SCALE,
                    )
                    pidx = kb - 4 * qb
                    if pidx >= 0:
                        nc.vector.tensor_mul(pt, pt, mask_sb[:, pidx, :])
                    return pt

                for h in range(HL):
                    pk0 = psyp.tile([128, 2, 256], f32, tag="psy",
                                    name=f"pk0_{qb}_{h}")
                    pk1 = psyp.tile([128, 2, 256], f32, tag="psy",
                                    name=f"pk1_{qb}_{h}")
                    psy = [pk0[:, 0, 0:D + 1], pk0[:, 1, 0:D + 1],
                           pk1[:, 0, 0:D + 1], pk1[:, 1, 0:D + 1]]
                    # software pipeline: S(kb+1)/exp are issued before PV(kb)
                    # so the in-order PE queue is never parked on the exp.
                    pt_next = s_exp(h, 0)
                    for kb in range(nkb):
                        pt = pt_next
                        if kb + 1 < nkb:
                            pt_next = s_exp(h, kb + 1)
                        for qs in range(4):
                            last = 4 * qb + qs
                            if kb <= last:
                                # Two accumulation chains share each PSUM
                                # bank: only the bank's FIRST matmul may
                                # carry start=True (it zero-marks the whole
                                # 2KB zero region); the second chain's
                                # region zero-fills on first touch via the
                                # per-element has_written bits.
                                nc.tensor.matmul(
                                    psy[qs],
                                    lhsT=pt[:, qs * 128:(qs + 1) * 128],
                                    rhs=v_nat[kb][:, h, :],
                                    start=(kb == 0 and qs % 2 == 0),
                                    stop=(kb == last),
                                )
                    ytr = pss.tile([128, 512], bf16, tag="s",
                                   name=f"ytr{qb}_{h}")
                    ytb = ytbp.tile([128, 512], bf16, tag="ytb",
                                    name=f"ytb{qb}_{h}")
                    for qs in range(4):
                        rec = ynp.tile([128, 1], f32, tag="rec",
                                       name=f"rec{qb}_{h}_{qs}")
                        nc.vector.reciprocal(rec, psy[qs][:, D:D + 1])
                        yn = ynp.tile([128, 128], bf16, tag="yn",
                                      name=f"yn{qb}_{h}_{qs}")
                        nc.vector.tensor_scalar_mul(yn, psy[qs][:, 0:D], rec)
                        nc.tensor.matmul(
                            ytr[:, qs * 128:(qs + 1) * 128],
                            lhsT=yn, rhs=identb,
                            is_transpose=True,
                        )
                    nc.vector.tensor_copy(ytb, ytr)
                    if qb < 3:
                        nc.sync.dma_start(
                            out=yag_in[qb][h * 128:(h + 1) * 128, :], in_=ytb)
                    else:
                        nc.sync.dma_start(
                            out=yag_in3[h // 2][(h % 2) * 128:(h % 2) * 128 + 128, :],
                            in_=ytb)
                    if after_h is not None:
                        after_h(h)

            def load_yag(qb):
                t = yagp.tile([128, 16, 512], bf16, tag="yag",
                              name=f"yag{qb}")
                if qb < 3:
                    nc.sync.dma_start(
                        out=t,
                        in_=yag_out[qb].rearrange("(gh p) n -> p gh n", p=128))
                else:
                    # global head gh = r*4 + h; split halves hold h 0-1 / 2-3.
                    # DMA APs are limited to 3 dims, so one DMA per local head.
                    tv = t.rearrange("p (r h) n -> p h r n", r=4)
                    for i in range(2):
                        sv = yag_out3[i].rearrange(
                            "(r h p) n -> p h r n", h=2, p=128)
                        for hh in range(2):
                            nc.sync.dma_start(
                                out=tv[:, 2 * i + hh, :, :],
                                in_=sv[:, hh, :, :])
                return t

            def proj_band(qb, yag_sb):
                q0 = qb * 512
                ota = outp.tile([128, 4, 512], f32, tag="ot", name=f"ot{qb}")
                for qc in range(4):
                    po = psmm.tile([128, 512], f32, tag="mm",
                                   name=f"po{qb}_{qc}")
                    for gh in range(16):
                        nc.tensor.matmul(
                            po,
                            lhsT=yag_sb[:, gh, qc * 128:(qc + 1) * 128],
                            rhs=wp_sb[:, gh, :],
                            start=(gh == 0),
                            stop=(gh == 15),
                        )
                    nc.scalar.copy(ota[:, qc, :], po)
                nc.sync.dma_start(
                    out=out_ext[q0:q0 + 512, :].rearrange(
                        "(qc p) n -> p qc n", p=128),
                    in_=ota,
                )

            for qq in range(4):
                tg = qq * QT
                xt = xt_cur
                if qq < 3:
                    xt_cur = load_x_quarter(qq + 1)
                if qq == 0:
                    nc.sync.dma_start(out=s2_sb[:, T // 2:T],
                                      in_=s2_in[:, T // 2:T])
                    nc.sync.dma_start(out=cos_sb[:, T // 2:T],
                                      in_=cos_in[:, T // 2:T])

                # k streams first (bands need k before v)
                for h in range(HL):
                    ps = psmm.tile([128, QT], f32, tag="mm", name=f"psk{qq}_{h}")
                    for cc in range(NCC):
                        nc.tensor.matmul(
                            ps, lhsT=wk_sb[:, h, cc, :], rhs=xt[:, cc, :],
                            start=(cc == 0), stop=(cc == NCC - 1),
                        )
                    rope_into(k_t[h][:, tg:tg + QT], ps, tg, f"k{qq}_{h}")
                # q streams
                q_cur = []
                for h in range(HL):
                    ps = psmm.tile([128, QT], f32, tag="mm", name=f"psq{qq}_{h}")
                    for cc in range(NCC):
                        nc.tensor.matmul(
                            ps, lhsT=wq_sb[:, h, cc, :], rhs=xt[:, cc, :],
                            start=(cc == 0), stop=(cc == NCC - 1),
                        )
                    qt = qp.tile([128, QT], bf16, tag=f"q{h}", name=f"qt{qq}_{h}")
                    rope_into(qt, ps, tg, f"q{qq}_{h}")
                    q_cur.append(qt)
                # v natural: x chunk stationary, all 4 heads in one sweep
                for tb in range(4):
                    psv = psmm.tile([128, W_LOC], f32, tag="mm",
                                    name=f"psv{qq}_{tb}")
                    for cc in range(NCC):
                        nc.tensor.matmul(
                            psv,
                            lhsT=xt[:, cc, tb * 128:(tb + 1) * 128],
                            rhs=wv_sb[:, cc, :],
                            start=(cc == 0), stop=(cc == NCC - 1),
                        )
                    vt = v_nat[qq * 4 + tb]
                    nc.vector.tensor_copy(
                        vt[:, :, 0:D],
                        psv.rearrange("p (a b) -> p a b", a=HL))

                if qq < 3:
                    band(qq, q_cur)
                    nc.gpsimd.collective_compute(
                        "AllGather", mybir.AluOpType.bypass,
                        replica_groups=GROUPS,
                        ins=[yag_in[qq].opt()], outs=[yag_out[qq].opt()],
                    )
                else:
                    # split band-3 gather: heads 0-1 go as soon as they're
                    # done so the tail AllGather mostly overlaps the band.
                    def after_h(h):
                        if h == 1:
                            nc.gpsimd.collective_compute(
                                "AllGather", mybir.AluOpType.bypass,
                                replica_groups=GROUPS,
                                ins=[yag_in3[0].opt()],
                                outs=[yag_out3[0].opt()],
                            )
                    band(3, q_cur, after_h=after_h)

                # wp and the yag prefetches are kept OUT of the AG0 window
                # (quarter 1) to avoid HBM contention with the collective;
                # each yag load sits AFTER an AG call so its conservative
                # collective-counter dependency is on a finished gather.
                if qq == 1:
                    wp_sb = constp.tile([128, 16, W_LOC], bf16, name="wp_sb")
                    nc.sync.dma_start(out=wp_sb, in_=wp_in[:, :, :])
                    yag0 = load_yag(0)
                if qq == 2:
                    yag1 = load_yag(1)

            nc.gpsimd.collective_compute(
                "AllGather", mybir.AluOpType.bypass, replica_groups=GROUPS,
                ins=[yag_in3[1].opt()], outs=[yag_out3[1].opt()],
            )
            yag2 = load_yag(2)
            proj_band(0, yag0)
            yag3 = load_yag(3)
            proj_band(1, yag1)
            proj_band(2, yag2)
            proj_band(3, yag3)

    nc.compile()
    _CACHE["nc"] = nc
    return nc


def _in_maps(x, Wqkv, Wproj):
    import ml_dtypes
    bf = ml_dtypes.bfloat16
    cos_t, _ = _host_tables()
    s2_t = _host_s2()
    masks = _host_masks()
    x = np.asarray(x, dtype=np.float32)
    Wqkv = np.asarray(Wqkv, dtype=np.float32)
    Wproj = np.asarray(Wproj, dtype=np.float32)
    maps = []
    for core in range(NCORES):
        b, r = divmod(core, 4)
        lo, hi = r * W_LOC, (r + 1) * W_LOC
        # x pre-staged per token-quarter: [qq, p, cc, t]
        xqa = x[b].reshape(4, QT, NCC, 128).transpose(0, 3, 2, 1)
        # weights pre-staged: [p, h, cc, n] / [p, cc, n]
        wq = Wqkv[:, lo:hi].reshape(NCC, 128, HL, 128).transpose(1, 2, 0, 3)
        wk = Wqkv[:, C + lo:C + hi].reshape(NCC, 128, HL, 128).transpose(1, 2, 0, 3)
        wv = Wqkv[:, 2 * C + lo:2 * C + hi].reshape(NCC, 128, W_LOC).transpose(1, 0, 2)
        wp = Wproj[:, lo:hi].reshape(16, 128, W_LOC).transpose(1, 0, 2)
        maps.append({
            "xq": np.ascontiguousarray(xqa).astype(bf),
            "wq": np.ascontiguousarray(wq).astype(bf),
            "wk": np.ascontiguousarray(wk).astype(bf),
            "wv": np.ascontiguousarray(wv).astype(bf),
            "wp": np.ascontiguousarray(wp).astype(bf),
            "cos_t": cos_t,
            "s2_t": s2_t,
            "masks": masks,
            "identb": np.eye(128, dtype=np.float32).astype(bf),
        })
    return maps


def _run(x, Wqkv, Wproj, trace=False, tmpdir=None):
    from concourse.bass_utils import run_bass_kernel_spmd
    nc = _build()
    maps = _in_maps(x, Wqkv, Wproj)
    res = run_bass_kernel_spmd(
        nc, maps, core_ids=list(range(NCORES)), trace=trace, tmpdir=tmpdir
    )
    out = np.empty((B, T, C), dtype=np.float32)
    for core in range(NCORES):
        b, r = divmod(core, 4)
        out[b, :, r * W_LOC:(r + 1) * W_LOC] = res.results[core]["out_shard"]
    return out, res


def kernel(x, Wqkv, Wproj):
    out, _ = _run(x, Wqkv, Wproj)
    return out


# revision 64
# speedup vs baseline: 1.0995x; 1.0995x over previous
"""Causal self-attention (RoPE) Trainium2 kernel.

Distribution: 8 cores = 2 data-parallel groups (batch dim, B=2) x 4
tensor-parallel cores (16 heads -> 4 heads/core).  Each core computes
QKV projection + RoPE + causal attention + output projection for its
batch and heads; an AllGather over each 4-core group shares y so each
core computes a 512-column shard of the output projection, which the
host reassembles.

Pipelined at token-quarter granularity: QKV(q) -> attention band(q) ->
AllGather(q), so collectives overlap compute and the PE never idles.
All bulk inputs are host-pre-staged into SBUF-native layouts so each
load is a single large contiguous DMA (sync-queue issue cost ~0.6us
per dma_start makes many small DMAs expensive).

Self-contained: hardcodes all shapes from the problem spec.
"""

import numpy as np

B, T, C = 2, 2048, 2048
H, D = 16, 128
HL = 4            # heads per core
W_LOC = HL * D    # 512 local head width
NCORES = 8
GROUPS = [[0, 1, 2, 3], [4, 5, 6, 7]]
SCALE = 1.0 / float(np.sqrt(D))
NCC = C // 128    # 16 contraction chunks
QT = 512          # token quarter

_CACHE = {}


def _host_tables():
    # Mirror reference _rope_tables in float32.
    inv_freq = (1.0 / (10000.0 ** (np.arange(0, D, 2, dtype=np.float32) / np.float32(D)))).astype(np.float32)
    t = np.arange(T, dtype=np.float32)
    freqs = np.outer(t, inv_freq).astype(np.float32)        # (T, D/2)
    emb = np.concatenate([freqs, freqs], axis=-1)           # (T, D)
    cos_t = np.ascontiguousarray(np.cos(emb).astype(np.float32).T)  # (D, T)
    sin_t = np.ascontiguousarray(np.sin(emb).astype(np.float32).T)
    return cos_t, sin_t


def _host_masks():
    import ml_dtypes
    # S^T-layout causal masks for the 4 diagonal phases, pre-staged as
    # [kk, phase, qq] so one DMA loads all four.
    # mask[p][kk, qq] = 1 if qq >= kk + p*128 else 0
    kk = np.arange(128)[:, None]
    qq = np.arange(512)[None, :]
    m = np.stack([(qq >= kk + p * 128) for p in range(4)]).astype(np.float32)
    m = np.ascontiguousarray(m.transpose(1, 0, 2))          # (128, 4, 512)
    return m.astype(ml_dtypes.bfloat16)


def _host_s2():
    # Signed/swap-ready sin table for rotate_half on the vector engine:
    # t1[d] = ps[(d+64)%128] * s2[d] with s2 = [-sin[0:64]; sin[64:128]].
    _, sin_t = _host_tables()
    s2 = sin_t.copy()
    s2[0:64, :] = -s2[0:64, :]
    return np.ascontiguousarray(s2)


def _build():
    if "nc" in _CACHE:
        return _CACHE["nc"]

    import concourse.mybir as mybir
    import concourse.tile as tile
    from concourse import bacc

    f32 = mybir.dt.float32
    f32r = mybir.dt.float32r
    bf16 = mybir.dt.bfloat16

    nc = bacc.Bacc(None, target_bir_lowering=False, num_devices=NCORES)

    # Host-pre-staged layouts: partition dim first, contiguous per line.
    xq = nc.dram_tensor("xq", [4, 128, NCC, QT], bf16, kind="ExternalInput")
    wk_in = nc.dram_tensor("wk", [128, HL, NCC, 128], bf16, kind="ExternalInput")
    wq_in = nc.dram_tensor("wq", [128, HL, NCC, 128], bf16, kind="ExternalInput")
    wv_in = nc.dram_tensor("wv", [128, NCC, W_LOC], bf16, kind="ExternalInput")
    wp_in = nc.dram_tensor("wp", [128, 16, W_LOC], bf16, kind="ExternalInput")
    cos_in = nc.dram_tensor("cos_t", [D, T], bf16, kind="ExternalInput")
    s2_in = nc.dram_tensor("s2_t", [D, T], bf16, kind="ExternalInput")
    masks = nc.dram_tensor("masks", [128, 4, 512], bf16, kind="ExternalInput")
    identb_in = nc.dram_tensor("identb", [128, 128], bf16, kind="ExternalInput")
    out_ext = nc.dram_tensor("out_shard", [T, W_LOC], f32, kind="ExternalOutput")

    with tile.TileContext(nc) as tc:
        with (
            tc.tile_pool(name="const", bufs=1) as constp,
            tc.tile_pool(name="pers", bufs=1) as pers,
            tc.tile_pool(name="xp", bufs=2) as xp,
            tc.tile_pool(name="qp", bufs=1) as qp,
            tc.tile_pool(name="ropet", bufs=2) as ropet,
            tc.tile_pool(name="ptp", bufs=3) as ptp,
            tc.tile_pool(name="ynp", bufs=2) as ynp,
            tc.tile_pool(name="ytbp", bufs=2) as ytbp,
            tc.tile_pool(name="yagp", bufs=3) as yagp,
            tc.tile_pool(name="outp", bufs=1) as outp,
            tc.tile_pool(name="psmm", bufs=3, space="PSUM") as psmm,
            tc.tile_pool(name="pss", bufs=2, space="PSUM") as pss,
            tc.tile_pool(name="psyp", bufs=3, space="PSUM") as psyp,
            tc.tile_pool(name="dram", bufs=1, space="DRAM") as dram,
        ):
            # bands 0-2 gather all 4 heads at once; band 3 gathers PER HEAD
            # so the tail AllGather is only 1/4-sized after the band ends.
            yag_in = [dram.tile([512, 512], bf16, name=f"yagin{qb}")
                      for qb in range(3)]
            yag_in3 = [dram.tile([128, 512], bf16, name=f"yagin3{i}")
                       for i in range(4)]
            yag_out = [dram.tile([2048, 512], bf16, name=f"yagout{qb}")
                       for qb in range(3)]
            yag_out3 = [dram.tile([512, 512], bf16, name=f"yagout3{i}")
                        for i in range(4)]

            def load_x_quarter(qq):
                t = xp.tile([128, NCC, QT], bf16, tag="xt", name=f"xt{qq}")
                # 4 sub-DMAs so the first chunks land (and matmuls start)
                # before the whole 2MB quarter transfers.
                for i in range(4):
                    nc.sync.dma_start(
                        out=t[:, 4 * i:4 * i + 4, :],
                        in_=xq[qq, :, 4 * i:4 * i + 4, :])
                return t

            # quarter-0 activations and k-weights first: the first compute
            # (k streams) becomes runnable as soon as these land.
            xt_cur = load_x_quarter(0)
            wk_sb = constp.tile([128, HL, NCC, 128], bf16)
            nc.sync.dma_start(out=wk_sb, in_=wk_in[:, :, :, :])
            wq_sb = constp.tile([128, HL, NCC, 128], bf16)
            nc.sync.dma_start(out=wq_sb, in_=wq_in[:, :, :, :])
            cos_sb = constp.tile([D, T], bf16)
            s2_sb = constp.tile([D, T], bf16)
            nc.sync.dma_start(out=s2_sb[:, 0:T // 2], in_=s2_in[:, 0:T // 2])
            nc.sync.dma_start(out=cos_sb[:, 0:T // 2], in_=cos_in[:, 0:T // 2])
            wv_sb = constp.tile([128, NCC, W_LOC], bf16)
            nc.sync.dma_start(out=wv_sb, in_=wv_in[:, :, :])
            mask_sb = constp.tile([128, 4, 512], bf16)
            nc.sync.dma_start(out=mask_sb, in_=masks[:, :, :])
            identb = constp.tile([128, 128], bf16)
            nc.sync.dma_start(out=identb, in_=identb_in[:, :])

            # Persistent activations: k^T full-T per head, v natural.
            k_t = []
            for h in range(HL):
                k_t.append(pers.tile([D, T], bf16, name=f"kt{h}"))
            v_nat = []
            for tt in range(T // 128):
                vt = pers.tile([128, HL, D + 1], bf16, name=f"vnat{tt}")
                nc.vector.memset(vt[:, :, D:D + 1], 1.0)
                v_nat.append(vt)

            def rope_into(dest, ps, tg, nm):
                # rotate_half on the DVE: two half-partition muls against the
                # sign-folded sin table (s2), keeping the PE free.
                t1 = ropet.tile([128, QT], bf16, tag="t1", name=f"t1_{nm}")
                nc.vector.tensor_mul(t1[0:64, :], ps[64:128, :],
                                     s2_sb[0:64, tg:tg + QT])
                nc.vector.tensor_mul(t1[64:128, :], ps[0:64, :],
                                     s2_sb[64:128, tg:tg + QT])
                t2 = ropet.tile([128, QT], bf16, tag="t2", name=f"t2_{nm}")
                nc.vector.tensor_mul(t2, ps, cos_sb[:, tg:tg + QT])
                nc.vector.tensor_add(dest, t2, t1)

            def band(qb, q_cur, after_h=None):
                nkb = 4 * (qb + 1)

                def s_exp(h, kb):
                    # columns < pidx*128 of a diagonal S^T tile are fully
                    # masked — skip computing them (matmul, exp, and mask
                    # all shrink to the live window [c0:512]).
                    pidx = kb - 4 * qb
                    c0 = pidx * 128 if pidx > 0 else 0
                    w = 512 - c0
                    ps_s = pss.tile([128, 512], f32, tag="s",
                                    name=f"s{qb}_{h}_{kb}")
                    nc.tensor.matmul(
                        ps_s[:, 0:w],
                        lhsT=k_t[h][:, kb * 128:(kb + 1) * 128],
                        rhs=q_cur[h][:, c0:512],
                    )
                    pt = ptp.tile([128, 512], bf16, tag="pt",
                                  name=f"pt{qb}_{h}_{kb}")
                    nc.scalar.activation(
                        pt[:, 0:w], ps_s[:, 0:w],
                        mybir.ActivationFunctionType.Exp,
                        scale=SCALE,
                    )
                    if pidx >= 0:
                        nc.vector.tensor_mul(
                            pt[:, 0:w], pt[:, 0:w],
                            mask_sb[:, pidx, c0:512])
                    return pt, c0

                for h in range(HL):
                    pk0 = psyp.tile([128, 2, 256], f32, tag="psy",
                                    name=f"pk0_{qb}_{h}")
                    pk1 = psyp.tile([128, 2, 256], f32, tag="psy",
                                    name=f"pk1_{qb}_{h}")
                    psy = [pk0[:, 0, 0:D + 1], pk0[:, 1, 0:D + 1],
                           pk1[:, 0, 0:D + 1], pk1[:, 1, 0:D + 1]]
                    # software pipeline: S(kb+1)/exp are issued before PV(kb)
                    # so the in-order PE queue is never parked on the exp.
                    pt_next, c0_next = s_exp(h, 0)
                    for kb in range(nkb):
                        pt, c0 = pt_next, c0_next
                        if kb + 1 < nkb:
                            pt_next, c0_next = s_exp(h, kb + 1)
                        for qs in range(4):
                            last = 4 * qb + qs
                            if kb <= last:
                                # Two accumulation chains share each PSUM
                                # bank: only the bank's FIRST matmul may
                                # carry start=True (it zero-marks the whole
                                # 2KB zero region); the second chain's
                                # region zero-fills on first touch via the
                                # per-element has_written bits.
                                nc.tensor.matmul(
                                    psy[qs],
                                    lhsT=pt[:, qs * 128 - c0:
                                            (qs + 1) * 128 - c0],
                                    rhs=v_nat[kb][:, h, :],
                                    start=(kb == 0 and qs % 2 == 0),
                                    stop=(kb == last),
                                )
                    ytr = pss.tile([128, 512], bf16, tag="s",
                                   name=f"ytr{qb}_{h}")
                    ytb = ytbp.tile([128, 512], bf16, tag="ytb",
                                    name=f"ytb{qb}_{h}")
                    for qs in range(4):
                        rec = ynp.tile([128, 1], f32, tag="rec",
                                       name=f"rec{qb}_{h}_{qs}")
                        nc.vector.reciprocal(rec, psy[qs][:, D:D + 1])
                        yn = ynp.tile([128, 128], bf16, tag="yn",
                                      name=f"yn{qb}_{h}_{qs}")
                        nc.scalar.mul(yn, psy[qs][:, 0:D], rec)
                        nc.tensor.matmul(
                            ytr[:, qs * 128:(qs + 1) * 128],
                            lhsT=yn, rhs=identb,
                            is_transpose=True,
                        )
                    nc.scalar.copy(ytb, ytr)
                    if qb < 3:
                        nc.sync.dma_start(
                            out=yag_in[qb][h * 128:(h + 1) * 128, :], in_=ytb)
                    else:
                        nc.sync.dma_start(out=yag_in3[h], in_=ytb)
                    if after_h is not None:
                        after_h(h)

            def load_yag(qb):
                t = yagp.tile([128, 16, 512], bf16, tag="yag",
                              name=f"yag{qb}")
                if qb < 3:
                    nc.sync.dma_start(
                        out=t,
                        in_=yag_out[qb].rearrange("(gh p) n -> p gh n", p=128))
                else:
                    # global head gh = r*4 + h; per-head gathers, so one
                    # 3-dim DMA per local head.
                    tv = t.rearrange("p (r h) n -> p h r n", r=4)
                    for hh in range(4):
                        nc.sync.dma_start(
                            out=tv[:, hh, :, :],
                            in_=yag_out3[hh].rearrange(
                                "(r p) n -> p r n", p=128))
                return t

            def proj_band(qb, yag_sb):
                q0 = qb * 512
                for qc in range(4):
                    po = psmm.tile([128, 512], f32, tag="mm",
                                   name=f"po{qb}_{qc}")
                    for gh in range(16):
                        nc.tensor.matmul(
                            po,
                            lhsT=yag_sb[:, gh, qc * 128:(qc + 1) * 128],
                            rhs=wp_sb[:, gh, :],
                            start=(gh == 0),
                            stop=(gh == 15),
                        )
                    otc = outp.tile([128, 512], f32, tag="ot",
                                    name=f"ot{qb}_{qc}")
                    nc.scalar.copy(otc, po)
                    nc.sync.dma_start(
                        out=out_ext[q0 + qc * 128:q0 + (qc + 1) * 128, :],
                        in_=otc,
                    )

            for qq in range(4):
                tg = qq * QT
                xt = xt_cur
                if qq < 3:
                    xt_cur = load_x_quarter(qq + 1)
                if qq >= 1:
                    nc.gpsimd.collective_compute(
                        "AllGather", mybir.AluOpType.bypass,
                        replica_groups=GROUPS,
                        ins=[yag_in[qq - 1].opt()],
                        outs=[yag_out[qq - 1].opt()],
                    )
                    if qq == 3:
                        # just after the AG2 call: conservative collective
                        # counter is 3 (AG2 included), not 7 — the load
                        # lands long before the projection needs it.
                        yag2 = load_yag(2)
                if qq == 0:
                    nc.sync.dma_start(out=s2_sb[:, T // 2:T],
                                      in_=s2_in[:, T // 2:T])
                    nc.sync.dma_start(out=cos_sb[:, T // 2:T],
                                      in_=cos_in[:, T // 2:T])

                # k streams first (bands need k before v)
                for h in range(HL):
                    ps = psmm.tile([128, QT], f32, tag="mm", name=f"psk{qq}_{h}")
                    for cc in range(NCC):
                        nc.tensor.matmul(
                            ps, lhsT=wk_sb[:, h, cc, :], rhs=xt[:, cc, :],
                            start=(cc == 0), stop=(cc == NCC - 1),
                        )
                    rope_into(k_t[h][:, tg:tg + QT], ps, tg, f"k{qq}_{h}")
                # q streams
                q_cur = []
                for h in range(HL):
                    ps = psmm.tile([128, QT], f32, tag="mm", name=f"psq{qq}_{h}")
                    for cc in range(NCC):
                        nc.tensor.matmul(
                            ps, lhsT=wq_sb[:, h, cc, :], rhs=xt[:, cc, :],
                            start=(cc == 0), stop=(cc == NCC - 1),
                        )
                    qt = qp.tile([128, QT], bf16, tag=f"q{h}", name=f"qt{qq}_{h}")
                    rope_into(qt, ps, tg, f"q{qq}_{h}")
                    q_cur.append(qt)
                # v natural: x chunk stationary, all 4 heads in one sweep
                for tb in range(4):
                    psv = psmm.tile([128, W_LOC], f32, tag="mm",
                                    name=f"psv{qq}_{tb}")
                    for cc in range(NCC):
                        nc.tensor.matmul(
                            psv,
                            lhsT=xt[:, cc, tb * 128:(tb + 1) * 128],
                            rhs=wv_sb[:, cc, :],
                            start=(cc == 0), stop=(cc == NCC - 1),
                        )
                    vt = v_nat[qq * 4 + tb]
                    nc.scalar.copy(
                        vt[:, :, 0:D],
                        psv.rearrange("p (a b) -> p a b", a=HL))

                if qq < 3:
                    band(qq, q_cur)
                else:
                    # per-head band-3 gathers: each head's y goes out as
                    # soon as it's done, so only a 1/4-sized AllGather
                    # trails the band.
                    def after_h(h):
                        nc.gpsimd.collective_compute(
                            "AllGather", mybir.AluOpType.bypass,
                            replica_groups=GROUPS,
                            ins=[yag_in3[h].opt()],
                            outs=[yag_out3[h].opt()],
                        )
                    band(3, q_cur, after_h=after_h)

                # A dma_start whose dependency is still pending holds its
                # rotating semaphore and can park a whole engine queue on
                # the sem-recycle wait — so every prefetch below is placed
                # where its dependency (collective counter) is ALREADY
                # satisfied when the sync queue reaches it.
                if qq == 1:
                    wp_sb = constp.tile([128, 16, W_LOC], bf16, name="wp_sb")
                    nc.sync.dma_start(out=wp_sb, in_=wp_in[:, :, :])
                    yag0 = load_yag(0)
                if qq == 2:
                    yag1 = load_yag(1)

            proj_band(0, yag0)
            yag3 = load_yag(3)    # slot A: WAR on proj0's reads
            proj_band(1, yag1)
            proj_band(2, yag2)
            proj_band(3, yag3)

    nc.compile()
    _CACHE["nc"] = nc
    return nc


def _in_maps(x, Wqkv, Wproj):
    import ml_dtypes
    bf = ml_dtypes.bfloat16
    cos_t, _ = _host_tables()
    s2_t = _host_s2()
    masks = _host_masks()
    x = np.asarray(x, dtype=np.float32)
    Wqkv = np.asarray(Wqkv, dtype=np.float32)
    Wproj = np.asarray(Wproj, dtype=np.float32)
    maps = []
    for core in range(NCORES):
        b, r = divmod(core, 4)
        lo, hi = r * W_LOC, (r + 1) * W_LOC
        # x pre-staged per token-quarter: [qq, p, cc, t]
        xqa = x[b].reshape(4, QT, NCC, 128).transpose(0, 3, 2, 1)
        # weights pre-staged: [p, h, cc, n] / [p, cc, n]
        wq = Wqkv[:, lo:hi].reshape(NCC, 128, HL, 128).transpose(1, 2, 0, 3)
        wk = Wqkv[:, C + lo:C + hi].reshape(NCC, 128, HL, 128).transpose(1, 2, 0, 3)
        wv = Wqkv[:, 2 * C + lo:2 * C + hi].reshape(NCC, 128, W_LOC).transpose(1, 0, 2)
        wp = Wproj[:, lo:hi].reshape(16, 128, W_LOC).transpose(1, 0, 2)
        maps.append({
            "xq": np.ascontiguousarray(xqa).astype(bf),
            "wq": np.ascontiguousarray(wq).astype(bf),
            "wk": np.ascontiguousarray(wk).astype(bf),
            "wv": np.ascontiguousarray(wv).astype(bf),
            "wp": np.ascontiguousarray(wp).astype(bf),
            "cos_t": cos_t.astype(bf),
            "s2_t": s2_t.astype(bf),
            "masks": masks,
            "identb": np.eye(128, dtype=np.float32).astype(bf),
        })
    return maps


def _run(x, Wqkv, Wproj, trace=False, tmpdir=None):
    from concourse.bass_utils import run_bass_kernel_spmd
    nc = _build()
    maps = _in_maps(x, Wqkv, Wproj)
    res = run_bass_kernel_spmd(
        nc, maps, core_ids=list(range(NCORES)), trace=trace, tmpdir=tmpdir
    )
    out = np.empty((B, T, C), dtype=np.float32)
    for core in range(NCORES):
        b, r = divmod(core, 4)
        out[b, :, r * W_LOC:(r + 1) * W_LOC] = res.results[core]["out_shard"]
    return out, res


def kernel(x, Wqkv, Wproj):
    out, _ = _run(x, Wqkv, Wproj)
    return out
